# revision 1
# baseline (speedup 1.0000x reference)
"""Trainium2 Bass kernel: NF4 dequant + linear, lean dequant pipeline.

y = x @ dequant(weight_q, absmax).T + bias

Sharding: column-parallel over out_features across 8 cores (1376 each).
Compute bf16 (f32 PSUM). Host preps, per core:
  xT    [4096, 8192] bf16 : x transposed (contraction on partitions)
  wq    [32,128,1376] bf16: NF4[q] * absmax  (fully dequantized weights)
Matmuls kt-outer / o-chunk-inner; token loop phased (A/B/C) so early PE
demand tracks the one-time weight DMA stream.
"""

import numpy as np
import ml_dtypes

import concourse.bacc as bacc
import concourse.mybir as mybir
import concourse.tile as tile
from concourse.alu_op_type import AluOpType
from concourse.bass_utils import run_bass_kernel_spmd

DT = mybir.dt

NF4 = np.array([
    -1.0, -0.6961928009986877, -0.5250730514526367, -0.39491748809814453,
    -0.28444138169288635, -0.18477343022823334, -0.09105003625154495, 0.0,
    0.07958029955625534, 0.16093020141124725, 0.24611230194568634,
    0.33791524171829224, 0.44070982933044434, 0.5626170039176941,
    0.7229568362236023, 1.0], dtype=np.float32)

P = 128
IN_F = 4096
OUT_F = 11008
N_CORES = 8
O_LOC = OUT_F // N_CORES          # 1376 out features per core
S_TOT = 4 * 2048                  # 8192 tokens
KT = IN_F // P                    # 32 contraction tiles
SP = 256                          # tokens per x macro tile (2 psum tiles)
NSP = S_TOT // SP                 # 32 x macro tiles
O_CHUNKS = [(1024, 352), (0, 512), (512, 512)]
PHASE_SPS = 14                    # x macro tiles in each phased (A/B/C) pass

_CACHE = {}


def _build():
    nc = bacc.Bacc()
    xT = nc.dram_tensor("xT", [IN_F, S_TOT], DT.bfloat16, kind="ExternalInput")
    wq_d = nc.dram_tensor("wq", [KT, P, O_LOC], DT.bfloat16,
                          kind="ExternalInput")
    biasb = nc.dram_tensor("biasb", [1, O_LOC], DT.float32,
                           kind="ExternalInput")
    y = nc.dram_tensor("y", [S_TOT, O_LOC], DT.float32, kind="ExternalOutput")

    with tile.TileContext(nc) as tc:
        with (
            tc.tile_pool(name="w", bufs=1) as wpool,
            tc.tile_pool(name="x", bufs=3) as xp,
            tc.tile_pool(name="o", bufs=4) as op,
            tc.tile_pool(name="ps", bufs=8, space="PSUM") as psp,
            tc.tile_pool(name="c", bufs=1) as cst,
        ):
            biasw = cst.tile([P, O_LOC], DT.float32)
            nc.gpsimd.dma_start(out=biasw[:],
                                in_=biasb[0, :].partition_broadcast(P))

            def load_x(sp):
                s0 = sp * SP
                xb = xp.tile([P, KT, SP], DT.bfloat16, tag="xb", name="xb")
                for g in range(4):
                    nc.sync.dma_start(
                        out=xb[:, g * 8:(g + 1) * 8, :],
                        in_=xT[g * 8 * P:(g + 1) * 8 * P, s0:s0 + SP]
                            .rearrange("(k p) s -> p k s", p=P))
                return xb

            xb_pre = [load_x(0)]

            # ---- weight chunks: straight DMA into persistent tiles.
            # oi-major = first-use order (phase A consumes oi 0 first).
            wt = {}
            qs = [nc.scalar, nc.gpsimd, nc.sync]
            for oi, (o0, osz) in enumerate(O_CHUNKS):
                for kt in range(KT):
                    w_t = wpool.tile([P, osz], DT.bfloat16, tag=f"w_{oi}_{kt}",
                                     name=f"w_{oi}_{kt}")
                    qs[oi % 3].dma_start(out=w_t[:],
                                         in_=wq_d[kt, :, o0:o0 + osz])
                    wt[(oi, kt)] = w_t

            def mm_block(sp, xb, ois):
                for half in range(2):
                    s0 = sp * SP + half * P
                    ps_ts = {oi: psp.tile([P, O_CHUNKS[oi][1]], DT.float32,
                                          tag="ps", name=f"ps_{sp}_{half}_{oi}")
                             for oi in ois}
                    sl = slice(half * P, (half + 1) * P)
                    for kt in range(KT):
                        for oi in ois:
                            nc.tensor.matmul(ps_ts[oi][:], xb[:, kt, sl],
                                             wt[(oi, kt)][:],
                                             start=(kt == 0),
                                             stop=(kt == KT - 1))
                    for oi in ois:
                        o0, osz = O_CHUNKS[oi]
                        out_t = op.tile([P, osz], DT.float32, tag="out",
                                        name="out_t")
                        nc.vector.tensor_tensor(out_t[:], ps_ts[oi][:],
                                                biasw[:, o0:o0 + osz],
                                                AluOpType.add)
                        nc.scalar.dma_start(out=y[s0:s0 + P, o0:o0 + osz],
                                            in_=out_t[:])

            # ---- phased token loop (x loads one iteration ahead) ----
            sched = ([(sp, [oi]) for oi in range(3)
                      for sp in range(PHASE_SPS)]
                     + [(sp, [0, 1, 2])
                        for sp in range(PHASE_SPS, NSP)])
            for idx, (sp, ois) in enumerate(sched):
                xb_cur = xb_pre.pop(0) if xb_pre else load_x(sp)
                if idx + 1 < len(sched):
                    xb_pre.append(load_x(sched[idx + 1][0]))
                mm_block(sp, xb_cur, ois)

    nc.compile()
    return nc


def _get_nc():
    if 'nc' not in _CACHE:
        _CACHE['nc'] = _build()
    return _CACHE['nc']


def make_in_maps(x, weight_q, absmax, bias):
    x = np.asarray(x, dtype=np.float32)
    weight_q = np.asarray(weight_q)
    absmax = np.asarray(absmax, dtype=np.float32)
    bias = np.asarray(bias, dtype=np.float32)
    bf16 = ml_dtypes.bfloat16

    xT = np.ascontiguousarray(x.reshape(S_TOT, IN_F).T.astype(bf16))
    in_maps = []
    for c in range(N_CORES):
        sl = slice(c * O_LOC, (c + 1) * O_LOC)
        q_c = np.ascontiguousarray(weight_q[sl].T)       # [4096, 1376] int32
        am = absmax[sl]                                  # [O_LOC, 64]
        am_exp = am.T.repeat(64, axis=0)                 # [4096, 1376] f32
        wq = (NF4[q_c] * am_exp).astype(bf16).reshape(KT, P, O_LOC)
        biasb_c = np.ascontiguousarray(bias[sl].reshape(1, O_LOC))
        in_maps.append({"xT": xT, "wq": np.ascontiguousarray(wq),
                        "biasb": biasb_c})
    return in_maps


def kernel(x, weight_q, absmax, bias):
    nc = _get_nc()
    in_maps = make_in_maps(x, weight_q, absmax, bias)
    res = run_bass_kernel_spmd(nc, in_maps, core_ids=list(range(N_CORES)))
    y = np.concatenate([res.results[c]["y"] for c in range(N_CORES)], axis=1)
    return np.ascontiguousarray(y.reshape(4, 2048, OUT_F))

